# revision 8
# baseline (speedup 1.0000x reference)
"""Trainium2 Bass kernel for nn_DiagGaussian (MoE routing).

Reference computation:
    action_mean[i] = x[i] @ W[index[i]] + b[index[i]]    # [B, O]
    action_std        = exp(logstd) broadcast to [B, O]

Sharding strategy (expert-parallel): core e owns expert e. The host
groups the batch rows by expert (argsort of index), pads each group to a
fixed capacity C, and ships core e the gathered rows (pre-transposed to
[I, C] so the contraction dim lands on SBUF partitions) plus only its
own W[e]/b[e]. Each core then runs one static dense GEMM - no on-device
routing at all - and the host scatters rows back to their original
positions. Padded output columns are discarded on the host, so pad
lanes are never zeroed.

The GEMM streams x in k-chunks so the PE trails the DMA stream by one
chunk group; DMA issue is spread across the Sync/Scalar/Vector
sequencers (a single sequencer issues DMA_DIRECT2D at only ~1.3/us);
outputs are written per chunk group as one large DMA. A short dummy-
matmul warmup keeps the PE HAM clock-gate at 2.4 GHz when real work
arrives.
"""

import math
import os
import sys
import types

import numpy as np


def _ensure_ntff_hook():
    """Make `antenv.axon_hooks` importable so run_bass_kernel_spmd(trace=True)
    can capture NTFF profiles under axon. The boot shim registers the hook only
    when the image ships an `antenv` package; this container doesn't, so we
    provide the two accessors ourselves, backed by the same ctypes hook."""
    try:
        import antenv.axon_hooks  # noqa: F401
        return
    except ImportError:
        pass
    hook = None
    try:
        from trn_agent_boot.trn_boot import _ntff_profile_via_ctypes
        hook = _ntff_profile_via_ctypes("/opt/axon/libaxon_pjrt.so")
    except Exception:
        pass
    pkg = sys.modules.get("antenv") or types.ModuleType("antenv")
    mod = types.ModuleType("antenv.axon_hooks")
    state = {"hook": hook}
    mod.get_axon_ntff_profile_hook = lambda: state["hook"]
    mod.set_axon_ntff_profile_hook = lambda h: state.__setitem__("hook", h)
    pkg.axon_hooks = mod
    sys.modules.setdefault("antenv", pkg)
    sys.modules["antenv.axon_hooks"] = mod


_ensure_ntff_hook()

import ml_dtypes
import concourse.mybir as mybir
import concourse.tile as tile
from concourse import bacc
from concourse.bass_utils import run_bass_kernel_spmd

B, I, O, E = 16384, 1024, 512, 8
NCORES = 8
KC = I // 128            # 8 contraction chunks of 128
CHUNK_RT = 6             # row-tiles (of 128 rows) per x DMA chunk group
C_DEFAULT = 2176         # per-expert row capacity (mean 2048 + 3 sigma;
                         # bumped at runtime in the unlikely overflow case)
WARMUP_MM = 12           # dummy matmuls to lift the PE HAM clock-gate

DEFAULT_MODE = os.environ.get("KERNEL_DTYPE", "bf16")

_PROGRAM_CACHE = {}
LAST_RESULTS = None      # BassKernelResults of the most recent run (for test harness)


def _build_program(C, mode):
    """Per-core program: y[C,O] = xT.T @ w + bias ; std = exp(logstd)."""
    assert C % 128 == 0
    RT = C // 128                         # row tiles
    groups = []                           # [(first_tile, n_tiles), ...]
    t = 0
    while t < RT:
        n = min(CHUNK_RT, RT - t)
        groups.append((t, n))
        t += n

    if mode == "bf16":
        xw_dt = mybir.dt.bfloat16
    elif mode == "f32r":
        xw_dt = mybir.dt.float32r
    else:
        raise ValueError(mode)

    nc = bacc.Bacc("TRN2", target_bir_lowering=False, debug=False, num_devices=NCORES)
    xT_d = nc.dram_tensor("xT", [I, C], xw_dt, kind="ExternalInput").ap()
    w_d = nc.dram_tensor("w", [I, O], xw_dt, kind="ExternalInput").ap()
    bias_d = nc.dram_tensor("bias", [128, O], mybir.dt.float32, kind="ExternalInput").ap()
    ls_d = nc.dram_tensor("ls", [1, O], mybir.dt.float32, kind="ExternalInput").ap()
    y_d = nc.dram_tensor("y", [C, O], mybir.dt.float32, kind="ExternalOutput").ap()
    std_d = nc.dram_tensor("std", [1, O], mybir.dt.float32, kind="ExternalOutput").ap()

    with tile.TileContext(nc) as tc:
        with (
            tc.tile_pool(name="weights", bufs=1) as wp,
            tc.tile_pool(name="xdata", bufs=KC * len(groups)) as xp,
            tc.tile_pool(name="youts", bufs=2) as op,
            tc.tile_pool(name="ps", bufs=4, space="PSUM") as ps,
            tc.tile_pool(name="pswarm", bufs=1, space="PSUM") as psw,
        ):
            # PE warmup: dummy matmuls on a zeroed tile keep the PE busy
            # during the input DMA lag so HAM reaches 2.4 GHz before real
            # matmuls issue (and the real stream never starts cold).
            warm_sb = wp.tile([128, O], mybir.dt.bfloat16)
            nc.gpsimd.memset(warm_sb[:], 0.0)
            warm_ps = psw.tile([128, O], mybir.dt.float32)
            for _ in range(WARMUP_MM):
                nc.tensor.matmul(
                    warm_ps[:], lhsT=warm_sb[:, :128], rhs=warm_sb[:],
                    start=True, stop=True,
                )

            # Resident weights: all 8 K-chunks side by side [128, 8*512].
            w_sb = wp.tile([128, KC * O], xw_dt)
            nc.sync.dma_start(
                out=w_sb[:].rearrange("p (k o) -> p k o", k=KC),
                in_=w_d.rearrange("(k p) o -> p k o", p=128),
            )
            bias_sb = wp.tile([128, O], mybir.dt.float32)
            nc.scalar.dma_start(out=bias_sb[:], in_=bias_d[:])

            # x chunks, issued group-major across two sequencers so the
            # first groups' operands arrive first and the PE streams
            # behind the DMA.
            xch = [[None] * len(groups) for _ in range(KC)]
            for g, (t0, nt) in enumerate(groups):
                for k in range(KC):
                    t = xp.tile([128, CHUNK_RT * 128], xw_dt,
                                name=f"xch_{k}_{g}", tag="xch")
                    eng = nc.sync if k % 2 == 0 else nc.scalar
                    eng.dma_start(
                        out=t[:, :nt * 128],
                        in_=xT_d[k * 128:(k + 1) * 128, t0 * 128:(t0 + nt) * 128],
                    )
                    xch[k][g] = t

            for g, (t0, nt) in enumerate(groups):
                yout = op.tile([128, CHUNK_RT * O], mybir.dt.float32,
                               name=f"yout_{g}", tag="yout")
                for j in range(nt):
                    psum = ps.tile([128, O], mybir.dt.float32)
                    for k in range(KC):
                        nc.tensor.matmul(
                            psum[:],
                            lhsT=xch[k][g][:, j * 128:(j + 1) * 128],
                            rhs=w_sb[:, k * O:(k + 1) * O],
                            start=(k == 0),
                            stop=(k == KC - 1),
                        )
                    nc.vector.tensor_add(
                        yout[:, j * O:(j + 1) * O], psum[:], bias_sb[:]
                    )
                nc.gpsimd.dma_start(
                    out=y_d[t0 * 128:(t0 + nt) * 128, :].rearrange(
                        "(j p) o -> p j o", p=128),
                    in_=yout[:, :nt * O].rearrange("p (j o) -> p j o", j=nt),
                )

            # exp(logstd) — tiny one-shot, emitted last so the ACT table
            # load doesn't sit in front of Scalar's DMA issues.
            ls_sb = wp.tile([1, O], mybir.dt.float32)
            std_sb = wp.tile([1, O], mybir.dt.float32)
            nc.scalar.dma_start(out=ls_sb[:], in_=ls_d[:])
            nc.scalar.activation(std_sb[:], ls_sb[:], mybir.ActivationFunctionType.Exp)
            nc.scalar.dma_start(out=std_d[:], in_=std_sb[:])

    nc.compile()
    return nc


def kernel(x, index, W, b, logstd, *, trace=False, mode=None):
    global LAST_RESULTS
    mode = mode or DEFAULT_MODE
    x = np.ascontiguousarray(x, dtype=np.float32)
    W = np.ascontiguousarray(W, dtype=np.float32)
    b = np.ascontiguousarray(b, dtype=np.float32)
    logstd = np.ascontiguousarray(logstd, dtype=np.float32)
    idx = np.asarray(index).astype(np.int64)

    order = np.argsort(idx, kind="stable")
    counts = np.bincount(idx, minlength=E)
    starts = np.zeros(E + 1, dtype=np.int64)
    np.cumsum(counts, out=starts[1:])

    C = max(C_DEFAULT, math.ceil(counts.max() / 128) * 128)
    key = (C, mode)
    nc = _PROGRAM_CACHE.get(key)
    if nc is None:
        nc = _PROGRAM_CACHE[key] = _build_program(C, mode)

    np_dt = ml_dtypes.bfloat16 if mode == "bf16" else np.float32
    in_maps = []
    for e in range(E):
        rows = order[starts[e]:starts[e + 1]]
        xgT = np.zeros((I, C), dtype=np_dt)
        xgT[:, :len(rows)] = x[rows].astype(np_dt).T
        in_maps.append({
            "xT": xgT,
            "w": W[e].astype(np_dt),
            "bias": np.ascontiguousarray(np.broadcast_to(b[e], (128, O))),
            "ls": logstd.reshape(1, O),
        })

    res = run_bass_kernel_spmd(nc, in_maps, list(range(NCORES)), trace=trace)
    LAST_RESULTS = res

    action_mean = np.empty((B, O), dtype=np.float32)
    for e in range(E):
        n = counts[e]
        action_mean[order[starts[e]:starts[e + 1]]] = res.results[e]["y"][:n]
    std_row = res.results[0]["std"][0]
    action_std = np.ascontiguousarray(np.broadcast_to(std_row, (B, O)))
    return action_mean, action_std


# revision 11
# speedup vs baseline: 1.3541x; 1.3541x over previous
"""Trainium2 Bass kernel for nn_DiagGaussian (MoE routing).

Reference computation:
    action_mean[i] = x[i] @ W[index[i]] + b[index[i]]    # [B, O]
    action_std        = exp(logstd) broadcast to [B, O]

Sharding strategy (expert-parallel): core e owns expert e. The host
groups the batch rows by expert (argsort of index), pads each group to a
fixed capacity C, and ships core e the gathered rows (pre-transposed to
[I, C] so the contraction dim lands on SBUF partitions) plus only its
own W[e]/b[e]. Each core then runs one static dense GEMM - no on-device
routing at all - and the host scatters rows back to their original
positions. Padded output columns are discarded on the host, so pad
lanes are never zeroed.

The GEMM streams x in k-chunks so the PE trails the DMA stream by one
chunk group; DMA issue is spread across the Sync/Scalar/Vector
sequencers (a single sequencer issues DMA_DIRECT2D at only ~1.3/us);
outputs are written per chunk group as one large DMA. A short dummy-
matmul warmup keeps the PE HAM clock-gate at 2.4 GHz when real work
arrives.
"""

import math
import os
import sys
import types

import numpy as np


def _ensure_ntff_hook():
    """Make `antenv.axon_hooks` importable so run_bass_kernel_spmd(trace=True)
    can capture NTFF profiles under axon. The boot shim registers the hook only
    when the image ships an `antenv` package; this container doesn't, so we
    provide the two accessors ourselves, backed by the same ctypes hook."""
    try:
        import antenv.axon_hooks  # noqa: F401
        return
    except ImportError:
        pass
    hook = None
    try:
        from trn_agent_boot.trn_boot import _ntff_profile_via_ctypes
        hook = _ntff_profile_via_ctypes("/opt/axon/libaxon_pjrt.so")
    except Exception:
        pass
    pkg = sys.modules.get("antenv") or types.ModuleType("antenv")
    mod = types.ModuleType("antenv.axon_hooks")
    state = {"hook": hook}
    mod.get_axon_ntff_profile_hook = lambda: state["hook"]
    mod.set_axon_ntff_profile_hook = lambda h: state.__setitem__("hook", h)
    pkg.axon_hooks = mod
    sys.modules.setdefault("antenv", pkg)
    sys.modules["antenv.axon_hooks"] = mod


_ensure_ntff_hook()

import ml_dtypes
import concourse.mybir as mybir
import concourse.tile as tile
from concourse import bacc
from concourse.bass_utils import run_bass_kernel_spmd

B, I, O, E = 16384, 1024, 512, 8
NCORES = 8
KC = I // 128            # 8 contraction chunks of 128
CHUNK_RT = 6             # row-tiles (of 128 rows) per x DMA chunk group
C_DEFAULT = 2176         # per-expert row capacity (mean 2048 + 3 sigma;
                         # bumped at runtime in the unlikely overflow case)
WARMUP_MM = 8           # dummy matmuls to lift the PE HAM clock-gate

DEFAULT_MODE = os.environ.get("KERNEL_DTYPE", "bf16")

_PROGRAM_CACHE = {}
LAST_RESULTS = None      # BassKernelResults of the most recent run (for test harness)


def _build_program(C, mode):
    """Per-core program: y[C,O] = xT.T @ w + bias ; std = exp(logstd)."""
    assert C % 128 == 0
    RT = C // 128                         # row tiles
    groups = []                           # [(first_tile, n_tiles), ...]
    t = 0
    while t < RT:
        n = min(CHUNK_RT, RT - t)
        groups.append((t, n))
        t += n

    if mode == "bf16":
        xw_dt = mybir.dt.bfloat16
    elif mode == "f32r":
        xw_dt = mybir.dt.float32r
    else:
        raise ValueError(mode)

    nc = bacc.Bacc("TRN2", target_bir_lowering=False, debug=False, num_devices=NCORES)
    xT_d = nc.dram_tensor("xT", [I, C], xw_dt, kind="ExternalInput").ap()
    w_d = nc.dram_tensor("w", [I, O], xw_dt, kind="ExternalInput").ap()
    bias_d = nc.dram_tensor("bias", [128, O], mybir.dt.float32, kind="ExternalInput").ap()
    ls_d = nc.dram_tensor("ls", [1, O], mybir.dt.float32, kind="ExternalInput").ap()
    y_d = nc.dram_tensor("y", [C, O], mybir.dt.float32, kind="ExternalOutput").ap()
    std_d = nc.dram_tensor("std", [1, O], mybir.dt.float32, kind="ExternalOutput").ap()

    with tile.TileContext(nc) as tc:
        with (
            tc.tile_pool(name="weights", bufs=1) as wp,
            tc.tile_pool(name="xdata", bufs=KC * len(groups)) as xp,
            tc.tile_pool(name="youts", bufs=4) as op,
            tc.tile_pool(name="ps", bufs=4, space="PSUM") as ps,
            tc.tile_pool(name="pswarm", bufs=1, space="PSUM") as psw,
        ):
            # PE warmup: dummy matmuls (on whatever the SBUF holds - the
            # results are never read) keep the PE busy during the input
            # DMA lag so HAM reaches 2.4 GHz before real matmuls issue.
            warm_sb = wp.tile([128, O], mybir.dt.bfloat16)
            nc.gpsimd.memset(warm_sb[:], 0.0)
            warm_ps = psw.tile([128, O], mybir.dt.float32)
            for _ in range(WARMUP_MM):
                nc.tensor.matmul(
                    warm_ps[:], lhsT=warm_sb[:, :128], rhs=warm_sb[:],
                    start=True, stop=True,
                )

            # Resident weights: all 8 K-chunks side by side [128, 8*512].
            # Split across the two HWDGE queues so the first chunk group
            # isn't serialized behind a single 1 MB transfer.
            w_sb = wp.tile([128, KC * O], xw_dt)
            half = KC // 2
            nc.sync.dma_start(
                out=w_sb[:, :half * O].rearrange("p (k o) -> p k o", k=half),
                in_=w_d[:half * 128].rearrange("(k p) o -> p k o", p=128),
            )
            nc.scalar.dma_start(
                out=w_sb[:, half * O:].rearrange("p (k o) -> p k o", k=KC - half),
                in_=w_d[half * 128:].rearrange("(k p) o -> p k o", p=128),
            )

            # x chunks, issued group-major across two sequencers so the
            # first groups' operands arrive first and the PE streams
            # behind the DMA.
            xch = [[None] * len(groups) for _ in range(KC)]
            for g, (t0, nt) in enumerate(groups):
                for k in range(KC):
                    t = xp.tile([128, CHUNK_RT * 128], xw_dt,
                                name=f"xch_{k}_{g}", tag="xch")
                    eng = nc.sync if k % 2 == 0 else nc.scalar
                    eng.dma_start(
                        out=t[:, :nt * 128],
                        in_=xT_d[k * 128:(k + 1) * 128, t0 * 128:(t0 + nt) * 128],
                    )
                    xch[k][g] = t

            # bias is first needed by the first psum evacuation (~15us in),
            # well after the first chunk group - issue it behind those.
            bias_sb = wp.tile([128, O], mybir.dt.float32)
            nc.scalar.dma_start(out=bias_sb[:], in_=bias_d[:])

            for g, (t0, nt) in enumerate(groups):
                for j in range(nt):
                    r = t0 + j
                    psum = ps.tile([128, O], mybir.dt.float32)
                    for k in range(KC):
                        nc.tensor.matmul(
                            psum[:],
                            lhsT=xch[k][g][:, j * 128:(j + 1) * 128],
                            rhs=w_sb[:, k * O:(k + 1) * O],
                            start=(k == 0),
                            stop=(k == KC - 1),
                        )
                    yout = op.tile([128, O], mybir.dt.float32,
                                   name=f"yout_{r}", tag="yout")
                    nc.vector.tensor_add(yout[:], psum[:], bias_sb[:])
                    eng = nc.gpsimd if r % 2 == 0 else nc.scalar
                    eng.dma_start(
                        out=y_d[r * 128:(r + 1) * 128, :], in_=yout[:]
                    )

            # exp(logstd) — tiny one-shot, emitted last so the ACT table
            # load doesn't sit in front of Scalar's DMA issues.
            ls_sb = wp.tile([1, O], mybir.dt.float32)
            std_sb = wp.tile([1, O], mybir.dt.float32)
            nc.scalar.dma_start(out=ls_sb[:], in_=ls_d[:])
            nc.scalar.activation(std_sb[:], ls_sb[:], mybir.ActivationFunctionType.Exp)
            nc.scalar.dma_start(out=std_d[:], in_=std_sb[:])

    nc.compile()
    return nc


def kernel(x, index, W, b, logstd, *, trace=False, mode=None):
    global LAST_RESULTS
    mode = mode or DEFAULT_MODE
    x = np.ascontiguousarray(x, dtype=np.float32)
    W = np.ascontiguousarray(W, dtype=np.float32)
    b = np.ascontiguousarray(b, dtype=np.float32)
    logstd = np.ascontiguousarray(logstd, dtype=np.float32)
    idx = np.asarray(index).astype(np.int64)

    order = np.argsort(idx, kind="stable")
    counts = np.bincount(idx, minlength=E)
    starts = np.zeros(E + 1, dtype=np.int64)
    np.cumsum(counts, out=starts[1:])

    C = max(C_DEFAULT, math.ceil(counts.max() / 128) * 128)
    key = (C, mode)
    nc = _PROGRAM_CACHE.get(key)
    if nc is None:
        nc = _PROGRAM_CACHE[key] = _build_program(C, mode)

    np_dt = ml_dtypes.bfloat16 if mode == "bf16" else np.float32
    in_maps = []
    for e in range(E):
        rows = order[starts[e]:starts[e + 1]]
        xgT = np.zeros((I, C), dtype=np_dt)
        xgT[:, :len(rows)] = x[rows].astype(np_dt).T
        in_maps.append({
            "xT": xgT,
            "w": W[e].astype(np_dt),
            "bias": np.ascontiguousarray(np.broadcast_to(b[e], (128, O))),
            "ls": logstd.reshape(1, O),
        })

    res = run_bass_kernel_spmd(nc, in_maps, list(range(NCORES)), trace=trace)
    LAST_RESULTS = res

    action_mean = np.empty((B, O), dtype=np.float32)
    for e in range(E):
        n = counts[e]
        action_mean[order[starts[e]:starts[e + 1]]] = res.results[e]["y"][:n]
    std_row = res.results[0]["std"][0]
    action_std = np.ascontiguousarray(np.broadcast_to(std_row, (B, O)))
    return action_mean, action_std
